# revision 85
# baseline (speedup 1.0000x reference)
"""Causal self-attention (B=2, S=2048, E=2048, H=16, D=128) with RoPE,
tensor-parallel over 8 TRN2 NeuronCores (2 heads per core).

Mixed fp16/fp8 compute with a PE-saturating cross-tile software pipeline:
- The three projection GEMMs (QKV, V, out) run on the PE in fp8e4
  DoubleRow mode (2 contraction k-tiles per instruction at 0.5
  cycles/row = 4x the fp16 MAC rate), using a 3-term hi/lo operand
  split (ah@bh + al@bh + ah@bl) that keeps ~fp16 accuracy at 75% of
  the fp16 cost. Operand planes are pre-scaled into e4m3's normal
  range on the host (subnormal underflow would otherwise destroy the
  lo-plane correction); the product scale is divided back out inside
  existing staging copies (Act copy-with-scale) or at host gather.
- Attention (scores, PV) stays fp16: score errors pass through exp and
  would dominate; DoubleRow would need a 2x64 contraction split with
  no net win at the required 2-term accuracy.
- Host: transpose x -> xh/xl [E, B*S] fp8 planes; per-core Wqkv/Wout
  hi/lo fp8 planes; RoPE cos fp16 and sin fp16 with the rotate-half
  signs folded in; identity + (-1e4 * lower-triangle) mask matrices.
- Device (per core, SPMD):
  * warmup matmuls on a memset tile hold the PE p-state ramp while the
    startup DMAs land; the first tile's projection is emitted in
    operand-arrival order against the serial DMA stream (hi*hi terms
    first, then the lo-plane correction terms).
  * QKV projection with weights stationary -> q,k in [D,S] layout; V
    with x stationary -> vT in [S,D] layout; rotate_half is a pair of
    strided SBUF->SBUF DMA copies (partition pair-swap) overlapped with
    PE work. Projection for tile i+1 is quartered and emitted as PE
    filler inside tile i's attention blocks.
  * scores transposed ([sk,sq] = k_chunk^T . q_block); exp'd probs feed
    P.V directly as the moving operand. Causal: above-diagonal chunks
    skipped; boundary chunks narrowed to their valid query columns, and
    the diagonal mask is folded in pre-exp by accumulating
    identity^T x (-1e4 * triangle) onto the scores so exp emits zeros.
  * softmax denominator: even/odd fp16 accumulators on DVE; the
    across-partition reduction + broadcast runs on the otherwise idle
    gpsimd engine, normalization deferred one block.
  * normalized context is emitted as scaled fp8 hi/lo planes (Act copy
    for hi, one gpsimd tensor_sub for lo) packed [P, 2, TT]
    so the out-projection contracts both heads per DoubleRow
    instruction; partials staged fp16, 4 output DMAs per tile.
- Host sums the 8 fp16 partials in f32, divides by the ctx*Wout plane
  scale, and transposes back.
"""

import numpy as np

import concourse.bass as bass
import concourse.bacc as bacc
import concourse.tile as tile
import concourse.mybir as mybir
import concourse.bass_isa as bass_isa
from concourse import bass_utils

B, S, E, H = 2, 2048, 2048, 16
D = E // H  # 128
NCORES = 8
HPC = H // NCORES  # heads per core = 2
T = B * S  # 4096 tokens
ROPE_BASE = 10000.0
P = 128
TT = 512  # token tile (free dim of most matmuls)
NTT = S // TT  # token tiles per batch = 4
NC_E = E // P  # contraction chunks over E = 16
FQKV = 3 * HPC * D  # per-core qkv features = 768
SCALE = float(D) ** -0.5
NWARM = 32  # warmup matmuls to hold the PE p-state ramp

# fp8 plane scales (powers of two). Chosen so every plane lands in
# e4m3's normal range: x ~ N(0,1) -> 32x; W ~ N(0, E^-1/2) -> 128x;
# ctx (convex combo of v ~ N(0,1)) -> 32x.
BX = 32.0
BW = 128.0
BC = 32.0
BWO = 128.0
SCL_P = 1.0 / (BX * BW)    # unscale for q/k/v staging copies
OUT_UNSCALE = 1.0 / (BC * BWO)  # host-side unscale of output partials

f16 = mybir.dt.float16
f8 = mybir.dt.float8e4
f32 = mybir.dt.float32
EXP = mybir.ActivationFunctionType.Exp
DR = mybir.MatmulPerfMode.DoubleRow
MUL = mybir.AluOpType.mult
SUB = mybir.AluOpType.subtract

SEQ_TILES = [(b, j) for b in range(B) for j in range(NTT)]


def _interleave(slots, fillers):
    """Yield from `slots`, spreading `fillers` (list of callables) evenly
    after slot index >= 1. Each yielded item is a callable."""
    n = len(slots)
    m = len(fillers)
    emitted = 0
    for i, s in enumerate(slots):
        yield s
        want = int(m * (i + 1) / n) if n else m
        while emitted < want:
            yield fillers[emitted]
            emitted += 1
    while emitted < m:
        yield fillers[emitted]
        emitted += 1


def _build_kernel(nc, tc, aps):
    (xhT, xlT, wqkvh, wqkvl, wouth, woutl, cosT, sinT, rt, idm, trim,
     trin, outT) = aps

    import contextlib
    ctx = contextlib.ExitStack()
    with ctx:
        ctx.enter_context(nc.allow_low_precision(
            reason="fp16/fp8 compute is intentional; tolerance is 2e-2"))
        const = ctx.enter_context(tc.tile_pool(name="const", bufs=1))
        sb = ctx.enter_context(tc.tile_pool(name="sb", bufs=2))
        ps = ctx.enter_context(tc.tile_pool(name="ps", bufs=1, space="PSUM"))

        # ---- constants + first-tile x ------------------------------------
        # DMA_ENGINES is a serial device in the cost model: issue order on
        # the two HWDGE queues (SP carries x, Act carries weights) is tuned
        # so the first projection's operands land first.
        ws_sb = const.tile([P, P], f16)
        nc.gpsimd.memset(ws_sb[:], 1.0)  # warmup operand, no DMA needed
        ws_bc = const.tile([P, P], f16)
        nc.gpsimd.memset(ws_bc[:], 1.0 / BC)  # final-reduce operand

        xs_t = {}  # tile idx -> (xh, xl) [P, NC_E, TT] fp8 planes

        def load_x(idx, splits, lo_splits=None, interleaved=False, lo_eng=None):
            b, j = SEQ_TILES[idx]
            xh = sb.tile([P, NC_E, TT], f8, tag="xh", bufs=3, name=f"xh_{b}_{j}")
            xl = sb.tile([P, NC_E, TT], f8, tag="xl", bufs=3, name=f"xl_{b}_{j}")
            xs_t[idx] = (xh, xl)
            col0 = b * S + j * TT
            xhr = xhT.rearrange("(c p) t -> p c t", p=P)
            xlr = xlT.rearrange("(c p) t -> p c t", p=P)
            if interleaved:
                # hi/lo chunk groups alternate so the first DoubleRow units
                # (which consume both planes of a chunk pair) start sooner.
                # Steady-state loads ride the Act queue so the small
                # latency-critical rotate-perm copies own the SP queue.
                c0 = 0
                for step in splits:
                    for xt, xr in ((xh, xhr), (xl, xlr)):
                        nc.scalar.dma_start(
                            xt[:, c0:c0 + step, :],
                            xr[:, c0:c0 + step, col0:col0 + TT])
                    c0 += step
                return
            for xt, xr, spl, eng in ((xh, xhr, splits, nc.sync),
                                     (xl, xlr, lo_splits or splits,
                                      lo_eng or nc.sync)):
                c0 = 0
                for step in spl:
                    eng.dma_start(
                        xt[:, c0:c0 + step, :],
                        xr[:, c0:c0 + step, col0:col0 + TT])
                    c0 += step

        def load_x_units(idx):
            """Prefetch issue units: one per chunk pair, riffled through a
            whole tile's emission so the DMA-device FIFO interleaves these
            bulk transfers with the small rotate-perm copies."""
            b, j = SEQ_TILES[idx]
            xh = sb.tile([P, NC_E, TT], f8, tag="xh", bufs=3, name=f"xh_{b}_{j}")
            xl = sb.tile([P, NC_E, TT], f8, tag="xl", bufs=3, name=f"xl_{b}_{j}")
            xs_t[idx] = (xh, xl)
            col0 = b * S + j * TT
            xhr = xhT.rearrange("(c p) t -> p c t", p=P)
            xlr = xlT.rearrange("(c p) t -> p c t", p=P)
            units = []
            for c0 in range(0, NC_E, 2):
                def u(c0=c0):
                    for xt, xr in ((xh, xhr), (xl, xlr)):
                        nc.scalar.dma_start(
                            xt[:, c0:c0 + 2, :],
                            xr[:, c0:c0 + 2, col0:col0 + TT])
                units.append(u)
            return units
            for xt, xr, spl, eng in ((xh, xhr, splits, nc.sync),
                                     (xl, xlr, lo_splits or splits, lo_eng or nc.sync)):
                c0 = 0
                for step in spl:
                    eng.dma_start(
                        xt[:, c0:c0 + step, :],
                        xr[:, c0:c0 + step, col0:col0 + TT])
                    c0 += step

        wq_h = const.tile([P, NC_E, FQKV], f8)
        wq_l = const.tile([P, NC_E, FQKV], f8)
        whr = wqkvh.rearrange("(c p) f -> p c f", p=P)
        wlr = wqkvl.rearrange("(c p) f -> p c f", p=P)
        wout_h = const.tile([P, HPC, E], f8)
        wout_l = const.tile([P, HPC, E], f8)
        wohr = wouth.rearrange("(hl p) e -> p hl e", p=P)
        wolr = woutl.rearrange("(hl p) e -> p hl e", p=P)
        cos_sb = const.tile([P, S], f16)
        sin_sb = const.tile([P, S], f16)
        id_sb = const.tile([P, P], f16)
        trim_sb = const.tile([P, P], f16)
        trin_sb = const.tile([P, P], f16)

        def wdma(wt, wr, cs):
            nc.scalar.dma_start(wt[:, cs, :], wr[:, cs, :])

        HC = NC_E // 2
        load_x(0, (4, 4, 4, 4), (4, 4, 4, 4))  # SP: x0 hi then lo quarters
        wdma(wq_h, whr, slice(0, 4))      # Act: W-hi chunks, full feature width
        wdma(wq_h, whr, slice(4, 8))
        wdma(wq_h, whr, slice(8, 12))
        wdma(wq_h, whr, slice(12, NC_E))
        wdma(wq_l, wlr, slice(0, HC))     # Act: W-lo halves
        wdma(wq_l, wlr, slice(HC, NC_E))
        # only tile 0's slice of cos/sin up front; the rest after x1
        nc.scalar.dma_start(cos_sb[:, :TT], cosT[:, :TT])
        nc.scalar.dma_start(sin_sb[:, :TT], sinT[:, :TT])
        nc.scalar.dma_start(id_sb[:], idm)
        nc.scalar.dma_start(trim_sb[:], trim)
        nc.scalar.dma_start(trin_sb[:], trin)

        def load_late_consts():
            nc.scalar.dma_start(cos_sb[:, TT:], cosT[:, TT:])
            nc.scalar.dma_start(sin_sb[:, TT:], sinT[:, TT:])
            nc.scalar.dma_start(wout_h[:], wohr)
            nc.scalar.dma_start(wout_l[:], wolr)

        # ---- warmup: keep PE continuously busy while DMAs land -----------
        def warm_group(n, pool_tag="mm", bufs=3):
            warm = ps.tile([P, TT], f32, tag=pool_tag, bufs=bufs, name="warm")
            for _ in range(n):
                nc.tensor.matmul(warm[:, :P], ws_sb[:], ws_sb[:], start=True,
                                 stop=True)

        warm_group(NWARM // 2)
        warm_group(NWARM - NWARM // 2)

        # ---- state -------------------------------------------------------
        qr_t = {}   # (b, hl, j) -> [P, TT] f16 roped q
        kr_t = {}   # (b, hl) -> [P, S] f16 roped k
        vt_t = {}   # b -> [P, NC_E, HPC*D] f16
        pctx_t = {}  # (b, hl, j) -> psum tile (unnormalized context)
        acc_t = {}  # (b, hl, j) -> (acc0, acc1) fp16 denominator partials
        ch_t = {}   # (b, j) -> [P, 2, TT] f8 scaled ctx hi planes
        cl_t = {}   # (b, j) -> [P, 2, TT] f8 scaled ctx lo planes

        def dr3(pm, w_pair_h, w_pair_l, m_pair_h, m_pair_l, first, last):
            """Emit the 3 hi/lo DoubleRow terms for one k-tile pair.
            Both hi-moving terms come first so a late lo plane (the ctx cl
            cast trails its ch by one engine hop) stalls only the last
            term."""
            nc.tensor.matmul(pm, w_pair_h, m_pair_h, start=first, stop=False,
                             perf_mode=DR)
            nc.tensor.matmul(pm, w_pair_l, m_pair_h, start=False, stop=False,
                             perf_mode=DR)
            nc.tensor.matmul(pm, w_pair_h, m_pair_l, start=False, stop=last,
                             perf_mode=DR)

        def emit_norm(blk):
            """Normalization for a finished block; writes the scaled fp8
            hi/lo context planes consumed by the DoubleRow out-projection."""
            bb, hl, j = blk
            acc0, acc1, lo1 = acc_t.pop(blk)

            def part1():
                accs = sb.tile([P, TT], f16, tag="accs", bufs=2)
                nc.vector.tensor_add(accs[:], acc0[:], acc1[:])
                den = sb.tile([P, TT], f16, tag="den", bufs=2)
                nc.gpsimd.partition_all_reduce(den[:], accs[:], P,
                                               bass_isa.ReduceOp.add)
                denb = sb.tile([P, TT], f16, tag="denb", bufs=2)
                nc.vector.tensor_scalar_mul(denb[:], den[:], 1.0 / BC)
                lbb = sb.tile([P, TT], f16, tag="lbb", bufs=2)
                nc.vector.reciprocal(lbb[:], denb[:])
                if hl == 0:
                    ch_t[(bb, j)] = sb.tile([P, HPC, TT], f8, tag="ch",
                                            bufs=3, name=f"ch_{bb}_{j}")
                    cl_t[(bb, j)] = sb.tile([P, HPC, TT], f8, tag="cl",
                                            bufs=3, name=f"cl_{bb}_{j}")
                # ctile = BC * ctx (the BC fold rode the reciprocal input);
                # ch is then a plain cast and cl a plain gpsimd subtract
                ctile = sb.tile([P, TT], f16, tag="ctx", bufs=3)
                pctx = pctx_t.pop(blk)
                nc.vector.tensor_mul(ctile[:], pctx[:], lbb[:])
                nc.scalar.copy(ch_t[(bb, j)][:, hl, :], ctile[:])
                nc.vector.tensor_sub(cl_t[(bb, j)][:, hl, :], ctile[:],
                                     ch_t[(bb, j)][:, hl, :])

            return [part1]

        # out-projection state: ost staging + DMA per `flush` ofs
        def make_po_pairs(src_tile_idx, flush=4, ost_eng=None, sc_from=99):
            """Return list of 16 callables, one per `of`; each does the
            3-term DoubleRow accumulation (both heads per instruction) +
            fp16 staging copy; every `flush`th flushes with one DMA."""
            bb, j = SEQ_TILES[src_tile_idx]
            cols = slice(bb * S + j * TT, bb * S + (j + 1) * TT)
            outr = outT.rearrange("(of p) t -> p of t", p=P)
            state = {}

            def pair(of):
                def emit():
                    if of % flush == 0:
                        state["ost"] = sb.tile([P, flush, TT], f16, tag="ost",
                                               bufs=3, name=f"ost_{bb}_{j}_{of}")
                    if of == sc_from:
                        # first reserved tail pair rides the free ctxp bank
                        # while the mm pool drains the last block's staging
                        po = ps.tile([P, TT], f32, tag="ctxp", bufs=2,
                                     name=f"po_{bb}_{j}_{of}")
                    else:
                        po = ps.tile([P, TT], f32, tag="mm", bufs=3,
                                     name=f"po_{bb}_{j}_{of}")
                    ofs = slice(of * P, (of + 1) * P)
                    dr3(po[:], wout_h[:, :, ofs], wout_l[:, :, ofs],
                        ch_t[(bb, j)][:], cl_t[(bb, j)][:], True, True)
                    # gpsimd cannot read PSUM: staging rides Act/DVE
                    eng = ost_eng or ("act" if of % 3 == 0 else "dve")
                    if eng == "act":
                        nc.scalar.copy(state["ost"][:, of % flush, :], po[:])
                    else:
                        nc.vector.tensor_copy(state["ost"][:, of % flush, :],
                                              po[:])
                    if of % flush == flush - 1:
                        of0 = of - (flush - 1)
                        nc.sync.dma_start(
                            outr[:, of0:of0 + flush, cols], state["ost"][:])
                        if of == 15:
                            ch_t.pop((bb, j))
                            cl_t.pop((bb, j))
                return emit
            return [pair(of) for of in range(16)]

        prev_blk = [None]  # block awaiting deferred normalization

        def attention_block(b, hl, j, fillers, pre=(), use_trim=False):
            """Emit one attention block; `fillers` are PE-filler callables
            (outproj pairs of the previous tile) spread through the chunk
            loop. Normalization of the previous block is emitted early."""
            qr = qr_t.pop((b, hl, j))
            kr = kr_t[(b, hl)]
            vt = vt_t[b]
            nch = 4 * j + 4
            cs_order = list(range(4 * j)) + [4 * j + r for r in range(4)]
            pctx = ps.tile([P, TT], f32, tag="ctxp", bufs=2,
                           name=f"pctx_{b}_{hl}_{j}")
            pctx_t[(b, hl, j)] = pctx
            acc0 = sb.tile([P, TT], f16, tag="acc", bufs=4)
            acc1 = sb.tile([P, TT], f16, tag="acc", bufs=4)
            # lo1: first valid column of acc1 (j=0 blocks start it narrow;
            # the head is zeroed so the combined denominator add is full)
            lo1 = P if j == 0 else 0
            if lo1:
                nc.vector.memset(acc1[:, :lo1], 0.0)
            acc_t[(b, hl, j)] = (acc0, acc1, lo1)
            ex_tiles = {}

            # boundary chunk r covers only query columns >= r*P
            def lo_of(i):
                r = cs_order[i] - 4 * j
                return max(r, 0) * P

            lag = min(4, nch)
            slots = []

            def psc_unit(i):
                def emit():
                    c = cs_order[i]
                    lo = lo_of(i)
                    w = TT - lo
                    boundary = c - 4 * j >= 0
                    psc = ps.tile([P, TT], f32, tag="sc", bufs=3)
                    nc.tensor.matmul(psc[:, :w], kr[:, c * P:(c + 1) * P],
                                     qr[:, lo:], start=True,
                                     stop=not (boundary and use_trim))
                    if boundary and use_trim:
                        # final block: fold the mask pre-exp so the tail's
                        # denominator chain never waits on the Pool engine
                        nc.tensor.matmul(psc[:, :P], id_sb[:], trin_sb[:],
                                         start=False, stop=True)
                    ex = sb.tile([P, TT], f16, tag="ex", bufs=6)
                    nc.scalar.activation(ex[:, :w], psc[:, :w], EXP,
                                         scale=SCALE)
                    if boundary and not use_trim:
                        # zero the above-diagonal entries of the 128
                        # diagonal columns on the idle gpsimd engine; the
                        # PV matmul over these columns is deferred to the
                        # end of the block so the PE never waits on it
                        nc.gpsimd.tensor_mul(ex[:, :P], ex[:, :P],
                                             trim_sb[:])
                    ex_tiles[i] = ex
                return emit

            deferred = []

            def pctx_unit(i):
                def emit():
                    c = cs_order[i]
                    lo = lo_of(i)
                    w = TT - lo
                    ex = ex_tiles.pop(i)
                    vt_s = vt[:, c, :]
                    vt_s = vt_s[:, hl * D:(hl + 1) * D]
                    if c - 4 * j >= 0 and not use_trim and i > 0:
                        # split boundary chunks: the masked 128 columns are
                        # deferred to block end (accumulate-only, start
                        # stays on chunk 0's full-width matmul so exactly
                        # one start=True write covers each PSUM region)
                        def masked_mm(stop):
                            nc.tensor.matmul(pctx[:, lo:lo + P], vt_s,
                                             ex[:, :P], start=False,
                                             stop=stop)
                        deferred.append(masked_mm)
                        if w > P:
                            nc.tensor.matmul(pctx[:, lo + P:], vt_s,
                                             ex[:, P:w], start=False,
                                             stop=False)
                    else:
                        nc.tensor.matmul(pctx[:, lo:], vt_s, ex[:, :w],
                                         start=(i == 0), stop=False)
                    acc = acc0 if i % 2 == 0 else acc1
                    if i < 2:
                        nc.vector.tensor_copy(acc[:, lo:], ex[:, :w])
                    else:
                        nc.vector.tensor_add(acc[:, lo:], acc[:, lo:],
                                             ex[:, :w])
                return emit

            for i in range(nch):
                slots.append(psc_unit(i))
                if i >= lag:
                    slots.append(pctx_unit(i - lag))
            for i in range(nch - lag, nch):
                slots.append(pctx_unit(i))

            # `pre` fillers run before the first score matmul to cover the
            # q-rope latency; they must NOT depend on the pending deferred
            # norm (projection units only, never out-proj pairs)
            rest = list(fillers)
            if prev_blk[0] is not None:
                part1, = emit_norm(prev_blk[0])
                emission = (list(pre) + [slots[0], part1] + slots[1:4] +
                            list(_interleave(slots[4:], rest)))
            else:
                emission = list(pre) + list(_interleave(slots, rest))
            for emit in emission:
                emit()
            for k, mm in enumerate(deferred):
                mm(stop=(k == len(deferred) - 1))
            prev_blk[0] = (b, hl, j)

        # ---- projection machinery ----------------------------------------
        # Projection for tile i+1 is emitted as quartered PE-filler units
        # inside tile i's attention blocks. All projection matmuls are fp8
        # DoubleRow 3-term: per chunk pair, hi*hi + lo*hi + hi*lo.

        def mk_rope(st, b, j, fb):
            def emit():
                is_q = fb < HPC
                hl = fb % HPC
                cs = slice(j * TT, (j + 1) * TT)
                raw = st["raw"].pop(fb)
                t1 = sb.tile([P, TT], f16, tag="rt1", bufs=2)
                nc.vector.tensor_mul(t1[:], raw[:], cos_sb[:, cs])
                if is_q:
                    qj = sb.tile([P, TT], f16, tag="qr", bufs=4,
                                 name=f"qr_{b}_{hl}_{j}")
                    qr_t[(b, hl, j)] = qj
                    dst = qj[:]
                else:
                    dst = kr_t[(b, hl)][:, cs]
                nc.vector.tensor_mul(dst, st["prot"].pop(fb)[:], sin_sb[:, cs])
                nc.vector.tensor_add(dst, dst, t1[:])
            return emit

        def mk_rot(st, fb):
            # rotate-half as an SBUF->SBUF partition pair-swap on the DMA
            # engines (2 strided copies) instead of a PE matmul; the signs
            # are folded into the host-built sin table
            def emit():
                raw = st["raw"][fb]
                praw = sb.tile([P, TT], f16, tag="praw", bufs=4,
                               name=f"praw_{fb}")
                nc.sync.dma_start(praw[0::2, :], raw[1::2, :])
                nc.sync.dma_start(praw[1::2, :], raw[0::2, :])
                st["prot"][fb] = praw
            return emit

        def proj_units_for(idx):
            b, j = SEQ_TILES[idx]
            xh, xl = xs_t.pop(idx)
            st = {"pmm": {}, "raw": {}, "prot": {}, "pv": {}}
            units = []

            if j == 0:
                def alloc_batch():
                    vt_t[b] = sb.tile([P, NC_E, HPC * D], f16, tag="vt",
                                      bufs=3, name=f"vt_{b}")
                    for hl in range(HPC):
                        kr_t[(b, hl)] = sb.tile([P, S], f16, tag="kr",
                                                bufs=4, name=f"kr_{b}_{hl}")
                units.append(alloc_batch)

            def qfb(fb, q):
                """2 chunk pairs x 3 DR terms for feature block fb."""
                def emit():
                    if q == 0:
                        st["pmm"][fb] = ps.tile([P, TT], f32, tag="mm",
                                                bufs=3, name=f"pmm_{fb}")
                    pmm = st["pmm"][fb]
                    fs = slice(fb * P, (fb + 1) * P)
                    for c in range(4 * q, 4 * q + 4, 2):
                        cp = slice(c, c + 2)
                        dr3(pmm[:], wq_h[:, cp, fs], wq_l[:, cp, fs],
                            xh[:, cp, :], xl[:, cp, :],
                            c == 0, c == NC_E - 2)
                    if q == 3:
                        raw = sb.tile([P, TT], f16, tag="raw", bufs=3)
                        nc.scalar.mul(raw[:], pmm[:], SCL_P)
                        st["raw"][fb] = raw
                return emit

            def vhalf(sub, h):
                """4 chunk pairs x 3 DR terms; x planes stationary."""
                def emit():
                    if h == 0:
                        st["pv"][sub] = ps.tile([P, TT], f32, tag="mm",
                                                bufs=3, name=f"pv_{sub}")
                    pv = st["pv"][sub]
                    ss = slice(sub * P, (sub + 1) * P)
                    for c in range(8 * h, 8 * h + 8, 2):
                        cp = slice(c, c + 2)
                        dr3(pv[:, :HPC * D], xh[:, cp, ss], xl[:, cp, ss],
                            wq_h[:, cp, 4 * P:], wq_l[:, cp, 4 * P:],
                            c == 0, c == NC_E - 2)
                    if h == 1:
                        nc.scalar.mul(vt_t[b][:, j * (TT // P) + sub, :],
                                      st["pv"].pop(sub)[:, :HPC * D], SCL_P)
                return emit

            # each fb's perm DMA (mk_rot) is issued right after its raw
            # staging, with its rope consumer several units later, so the
            # SBUF->SBUF pair-swap latency hides behind other PE work
            units += [qfb(0, q) for q in range(4)]
            units.append(mk_rot(st, 0))
            units += [qfb(1, q) for q in range(4)]
            units.append(mk_rot(st, 1))
            units += [qfb(2, q) for q in range(4)]
            units.append(mk_rot(st, 2))
            units.append(mk_rope(st, b, j, 0))
            units += [qfb(3, q) for q in range(4)]
            units.append(mk_rot(st, 3))
            units.append(mk_rope(st, b, j, 1))
            units += [vhalf(0, 0), vhalf(0, 1),
                      mk_rope(st, b, j, 2), vhalf(1, 0), vhalf(1, 1),
                      mk_rope(st, b, j, 3), vhalf(2, 0), vhalf(2, 1),
                      vhalf(3, 0), vhalf(3, 1)]
            return units

        def _riffle(a, b_):
            """Merge two callable lists proportionally, preserving order."""
            out = []
            na, nb = len(a), len(b_)
            ia = ib = 0
            for k in range(na + nb):
                if ia * (nb or 1) * 1.0 <= ib * (na or 1) and ia < na or ib >= nb:
                    out.append(a[ia]); ia += 1
                else:
                    out.append(b_[ib]); ib += 1
            return out

        # ---- tile 0 projection: standalone, paced by the startup DMAs ----
        b0, j0 = SEQ_TILES[0]
        xh0, xl0 = xs_t.pop(0)
        vt_t[b0] = sb.tile([P, NC_E, HPC * D], f16, tag="vt", bufs=3,
                           name="vt_0")
        for hl in range(HPC):
            kr_t[(b0, hl)] = sb.tile([P, S], f16, tag="kr", bufs=4,
                                     name=f"kr_0_{hl}")
        st0 = {"pmm": {}, "raw": {}, "prot": {}}
        for fb in range(4):
            tag = "mm" if fb < 2 else "sc"
            st0["pmm"][fb] = ps.tile([P, TT], f32, tag=tag, bufs=3,
                                     name=f"pmm0_{fb}")

        def t0_term(fb, q, w_t, m_t, first, last):
            fs = slice(fb * P, (fb + 1) * P)
            for c in range(4 * q, 4 * q + 4, 2):
                cp = slice(c, c + 2)
                nc.tensor.matmul(st0["pmm"][fb][:], w_t[:, cp, fs],
                                 m_t[:, cp, :],
                                 start=(first and c == 4 * q),
                                 stop=(last and c == 4 * q + 2),
                                 perf_mode=DR)

        # phase A: hi*hi terms in DMA-arrival order; B: lo-x; C: lo-w.
        # warm groups pace the PE against the serial DMA stream.
        groups = [(0, 0), "W", (1, 0), "W", (0, 1), (1, 1), "W", (2, 0),
                  (3, 0), "W", (0, 2), (1, 2), "W", (2, 1), (3, 1), "W",
                  (0, 3), (1, 3), "W", (2, 2), (3, 2), "W", (2, 3), (3, 3)]
        for g in groups:
            if g == "W":
                warm_group(4, "ctxp", bufs=2)
                continue
            fb, q = g
            t0_term(fb, q, wq_h, xh0, q == 0, False)
        for q in range(4):
            warm_group(3, "ctxp", bufs=2)
            for fb in range(4):
                t0_term(fb, q, wq_h, xl0, False, False)
        for q in range(4):
            if q < 2:
                warm_group(3, "ctxp", bufs=2)
            for fb in range(4):
                t0_term(fb, q, wq_l, xh0, False, q == 3)
        for fb in range(4):
            raw = sb.tile([P, TT], f16, tag="raw", bufs=3)
            nc.scalar.mul(raw[:], st0["pmm"][fb][:], SCL_P)
            st0["raw"][fb] = raw

        def v_sub0(sub):
            pv = ps.tile([P, TT], f32, tag="mm", bufs=3, name=f"pv0_{sub}")
            ss = slice(sub * P, (sub + 1) * P)
            for c in range(0, NC_E, 2):
                cp = slice(c, c + 2)
                dr3(pv[:, :HPC * D], xh0[:, cp, ss], xl0[:, cp, ss],
                    wq_h[:, cp, 4 * P:], wq_l[:, cp, 4 * P:],
                    c == 0, c == NC_E - 2)
            nc.scalar.mul(vt_t[b0][:, sub, :], pv[:, :HPC * D], SCL_P)

        mk_rot(st0, 0)()
        mk_rot(st0, 1)()
        mk_rope(st0, 0, 0, 0)()
        mk_rot(st0, 2)()
        mk_rope(st0, 0, 0, 1)()
        v_sub0(0)
        mk_rot(st0, 3)()
        mk_rope(st0, 0, 0, 2)()
        v_sub0(1)
        mk_rope(st0, 0, 0, 3)()
        v_sub0(2)
        v_sub0(3)
        load_x(1, (8, 8), interleaved=True)
        load_late_consts()

        # ---- main pipeline: attention(i) x projection(i+1) ----------------
        def warm_unit():
            def emit():
                warm_group(4, "ctxp", bufs=2)
            return emit

        tail_pairs = []  # po pairs reserved as PE cover for the tail norm
        for idx, (b, j) in enumerate(SEQ_TILES):
            pu = proj_units_for(idx + 1) if idx + 1 < len(SEQ_TILES) else []
            last = idx == len(SEQ_TILES) - 1
            pairs = (make_po_pairs(idx - 1, sc_from=8 if last else 99)
                     if idx >= 1 else [])
            half = 8
            if last:
                pairs, tail_pairs = pairs[:8], pairs[8:]
                half = 4
            h = len(pu) // 2
            attention_block(b, 0, j, _riffle(pairs[:half], pu[3:h]),
                            pre=pu[:3], use_trim=(j == 0))
            # issue the x prefetch at the block boundary: the Act queue has
            # an exp lull here, so the 4 dma issues don't delay the chunk
            # pipeline the way a loop-head burst does
            if idx + 2 < len(SEQ_TILES):
                load_x(idx + 2, (8, 8), interleaved=True)
            # the hl=1 block's pending norm writes this tile's ctx planes,
            # not the previous tile's, so po pairs are safe pre-roll there
            pre1, rest1 = ((pairs[half:half + 3], pairs[half + 3:])
                           if not pu else (pu[h:h + 3], pairs[half:]))
            attention_block(b, 1, j, _riffle(rest1, pu[h + 3:]),
                            pre=pre1,
                            use_trim=(j == 0 or idx == len(SEQ_TILES) - 1))

        # ---- tail: last block's norm + last tile's outproj. Both PSUM
        # pools are idle now: hold many banks so the drain is short.
        # The final block's denominator reduction runs on the PE (ones^T @
        # accs broadcasts column sums) instead of the gpsimd engine: the
        # whole tail waits on this chain, and the PE path is ~1.5us shorter.
        fb_, fhl, fj = prev_blk[0]
        facc0, facc1, _ = acc_t.pop(prev_blk[0])
        faccs = sb.tile([P, TT], f16, tag="accs", bufs=2)
        nc.vector.tensor_add(faccs[:], facc0[:], facc1[:])
        fden = ps.tile([P, TT], f32, tag="sc", bufs=3, name="fden")
        nc.tensor.matmul(fden[:], ws_bc[:], faccs[:], start=True, stop=True)
        # reserved out-proj pairs of the previous tile: in-order PE work
        # emitted right after the reduce so the whole recip/cast chain of
        # the final norm is covered by real matmuls
        for f in tail_pairs:
            f()
        flbb = sb.tile([P, TT], f16, tag="lbb", bufs=2)
        nc.vector.reciprocal(flbb[:], fden[:])
        fpctx = pctx_t.pop(prev_blk[0])
        # fp8 hi plane in one DVE hop from PSUM (skips the Act cast on the
        # tail critical path); the fp16 stage only feeds the lo-plane sub,
        # which also rides DVE here (the gpsimd Q7 launch would put ~1.2us
        # back on the critical path)
        nc.vector.tensor_mul(ch_t[(fb_, fj)][:, fhl, :], fpctx[:], flbb[:])
        fctile = sb.tile([P, TT], f16, tag="ctx", bufs=3)
        nc.vector.tensor_mul(fctile[:], fpctx[:], flbb[:])
        nc.vector.tensor_sub(cl_t[(fb_, fj)][:, fhl, :], fctile[:],
                             ch_t[(fb_, fj)][:, fhl, :])
        bb, j = SEQ_TILES[-1]
        cols = slice(bb * S + j * TT, bb * S + (j + 1) * TT)
        outr = outT.rearrange("(of p) t -> p of t", p=P)
        tpo = {}
        ost_t = {}

        def t_alloc(of):
            if of < 7:
                if of == 6:
                    tag, bufs = "ctxp", 2
                else:
                    tag, bufs = ("mm" if of % 2 == 0 else "sc"), 3
                tpo[of] = ps.tile([P, TT], f32, tag=tag, bufs=bufs,
                                  name=f"tpo_{of}")
            else:
                tpo[of] = ps.tile([P, TT], f32, tag="mm", bufs=3,
                                  name=f"tpo_{of}")

        def t_start(of):
            # hi*hi term only: depends on ch but not cl, so these run
            # while the gpsimd engine is still producing cl
            ofs = slice(of * P, (of + 1) * P)
            nc.tensor.matmul(tpo[of][:], wout_h[:, :, ofs],
                             ch_t[(bb, j)][:], start=True, stop=False,
                             perf_mode=DR)

        def t_rest(of):
            ofs = slice(of * P, (of + 1) * P)
            nc.tensor.matmul(tpo[of][:], wout_l[:, :, ofs],
                             ch_t[(bb, j)][:], start=False, stop=False,
                             perf_mode=DR)
            nc.tensor.matmul(tpo[of][:], wout_h[:, :, ofs],
                             cl_t[(bb, j)][:], start=False, stop=True,
                             perf_mode=DR)

        def t_copy(of):
            if of % 2 == 0:
                ost_t[of // 2] = sb.tile([P, 2, TT], f16, tag="tost", bufs=6,
                                         name=f"tost_{of}")
            if of % 2 == 0:
                nc.scalar.copy(ost_t[of // 2][:, of % 2, :], tpo.pop(of)[:])
            else:
                nc.vector.tensor_copy(ost_t[of // 2][:, of % 2, :],
                                      tpo.pop(of)[:])
            if of % 2 == 1:
                nc.sync.dma_start(outr[:, of - 1:of + 1, cols],
                                  ost_t.pop(of // 2)[:])

        for of in range(7):
            t_alloc(of)
            t_start(of)
        for of in range(7):
            t_rest(of)
            t_copy(of)
        for of in range(7, 14):
            t_alloc(of)
            t_start(of)
            t_rest(of)
            t_copy(of)
        # final ofs flush one at a time so the drain waits on the
        # smallest possible last DMA
        for of in (14, 15):
            t_alloc(of)
            t_start(of)
            t_rest(of)
            ost1 = sb.tile([P, TT], f16, tag="tost", bufs=6,
                           name=f"tost1_{of}")
            if of == 14:
                nc.vector.tensor_copy(ost1[:], tpo.pop(of)[:])
            else:
                nc.scalar.copy(ost1[:], tpo.pop(of)[:])
            nc.sync.dma_start(outr[:, of, cols], ost1[:])
        ch_t.pop((bb, j))
        cl_t.pop((bb, j))


def build_nc():
    nc = bacc.Bacc("TRN2", target_bir_lowering=False, debug=False,
                   num_devices=NCORES)
    xhT = nc.dram_tensor("xhT", [E, T], f8, kind="ExternalInput").ap()
    xlT = nc.dram_tensor("xlT", [E, T], f8, kind="ExternalInput").ap()
    wqkvh = nc.dram_tensor("wqkvh", [E, FQKV], f8, kind="ExternalInput").ap()
    wqkvl = nc.dram_tensor("wqkvl", [E, FQKV], f8, kind="ExternalInput").ap()
    wouth = nc.dram_tensor("wouth", [HPC * D, E], f8,
                           kind="ExternalInput").ap()
    woutl = nc.dram_tensor("woutl", [HPC * D, E], f8,
                           kind="ExternalInput").ap()
    cosT = nc.dram_tensor("cosT", [D, S], f16, kind="ExternalInput").ap()
    sinT = nc.dram_tensor("sinT", [D, S], f16, kind="ExternalInput").ap()
    rt = nc.dram_tensor("rt", [P, P], f16, kind="ExternalInput").ap()
    idm = nc.dram_tensor("idm", [P, P], f16, kind="ExternalInput").ap()
    trim = nc.dram_tensor("trim", [P, P], f16, kind="ExternalInput").ap()
    trin = nc.dram_tensor("trin", [P, P], f16, kind="ExternalInput").ap()
    outT = nc.dram_tensor("outT", [E, T], f16, kind="ExternalOutput").ap()
    with tile.TileContext(nc) as tc:
        _build_kernel(nc, tc, (xhT, xlT, wqkvh, wqkvl, wouth, woutl,
                               cosT, sinT, rt, idm, trim, trin, outT))
    nc.compile()
    return nc


def _hilo(a32):
    """Split a (pre-scaled) f32 array into e4m3 hi/lo planes."""
    import ml_dtypes
    e4 = ml_dtypes.float8_e4m3
    hi = a32.astype(e4)
    lo = (a32 - hi.astype(np.float32)).astype(e4)
    return hi, lo


def host_inputs(x, Wqkv, Wout):
    """Per-core input dicts (numpy fp16/fp8)."""
    xT = np.ascontiguousarray(x.reshape(T, E).T).astype(np.float32)
    xhT, xlT = _hilo(xT * BX)

    inv_freq = 1.0 / (ROPE_BASE ** (np.arange(0, D, 2, dtype=np.float64) / D))
    pos = np.arange(S, dtype=np.float64)
    freqs = np.outer(pos, inv_freq)            # [S, D/2]
    ang = np.concatenate([freqs, freqs], -1)   # [S, D]
    cosT = np.ascontiguousarray(np.cos(ang).T).astype(np.float16)
    # rotate-half folded into the sin table: perm(q)[2m] = q[2m+1] needs
    # -sin on even rows, +sin on odd rows
    sgn = np.where(np.arange(D)[:, None] % 2 == 0, -1.0, 1.0)
    sinT = np.ascontiguousarray(np.sin(ang).T * sgn).astype(np.float16)
    # pure pair-swap permutation (signs live in sinT), used by tile 0's
    # PE-matmul rotate; steady tiles use DMA pair-swaps instead
    rt = np.zeros((P, P), np.float16)
    for m in range(P // 2):
        rt[2 * m + 1, 2 * m] = 1.0
        rt[2 * m, 2 * m + 1] = 1.0

    idm = np.eye(P, dtype=np.float16)
    pp = np.arange(P)[:, None]
    ff = np.arange(P)[None, :]
    trim = (pp <= ff).astype(np.float16)
    trin = np.where(pp > ff, -1e4, 0.0).astype(np.float16)

    in_maps = []
    for c in range(NCORES):
        r0 = HPC * D * c  # 256*c
        wq = Wqkv[r0:r0 + HPC * D]
        wk = Wqkv[E + r0:E + r0 + HPC * D]
        wv = Wqkv[2 * E + r0:2 * E + r0 + HPC * D]
        wqkvT = np.ascontiguousarray(
            np.concatenate([wq, wk, wv], 0).T).astype(np.float32)
        wqkvh, wqkvl = _hilo(wqkvT * BW)
        woutT = np.ascontiguousarray(
            Wout[:, r0:r0 + HPC * D].T).astype(np.float32)
        wouth, woutl = _hilo(woutT * BWO)
        in_maps.append({
            "xhT": xhT, "xlT": xlT, "wqkvh": wqkvh, "wqkvl": wqkvl,
            "wouth": wouth, "woutl": woutl,
            "cosT": cosT, "sinT": sinT, "rt": rt, "idm": idm,
            "trim": trim, "trin": trin,
        })
    return in_maps


_NC_CACHE = None


def kernel(x, Wqkv, Wout):
    global _NC_CACHE
    x = np.asarray(x)
    Wqkv = np.asarray(Wqkv)
    Wout = np.asarray(Wout)
    in_maps = host_inputs(x, Wqkv, Wout)
    if _NC_CACHE is None:
        _NC_CACHE = build_nc()
    res = bass_utils.run_bass_kernel_spmd(
        _NC_CACHE, in_maps, core_ids=list(range(NCORES)))
    acc = np.zeros((E, T), np.float32)
    for c in range(NCORES):
        acc += res.results[c]["outT"].astype(np.float32)
    out = (acc * OUT_UNSCALE).T.reshape(B, S, E).astype(np.float32)
    return out
